# revision 31
# baseline (speedup 1.0000x reference)
"""Channel self-attention module (CSMA) on 8 Trainium2 NeuronCores.

Math: with x [B,C,N,H,W], C==HID==OUT==128, L=N*H*W, the module is
    q = Wq x + bq ; k = Wk x + bk ; v = Wv x + bv          (per-batch [C,L])
    A = softmax(q k^T)                                     ([C,C], rows)
    out = Wo (A v) + bo + x ; result = mean_N(out)         ([C,H*W])

Everything except the softmax is linear in x, so per batch only two small
sufficient statistics of x are needed:
    G = x x^T  [C,C]   and   s = x 1_L  [C]
    logits = Wq G Wk^T + (Wq s) bk^T + bq (Wk s)^T + L bq bk^T
    A = softmax(logits)
    result = (Wo A Wv + I) x_mean + (Wo A bv + bo)
where x_mean = mean over N of x (shape [C, H*W]).

The input stream (13 MB/core fp16) saturates the per-core HBM share
(~358 GB/s) and is the wall-clock floor, so the schedule keeps all other
engines hidden underneath it: x is pre-transposed on the host to l-major
fp16 chunks [128l, 130] (128 cols + ones + pad) and DMAd into SBUF fully
resident (no buffer recycling -> DMA never stalls on compute). Per tile,
PE accumulates [G|s]; DVE folds groups of 4 chunks that share an x_mean
window (k, k+98, k+196, k+294 -- 98*128 = 4*3136) in two add rounds; PE
transposes each folded chunk into the x_mean PSUM window via an I/16
matmul (one tile behind the G stream). The host orders fold groups by
PSUM bank so banks complete early and drain mid-stream (ACT/DVE
alternating). Pass 2 is a short serial tail of [128,128]-scale fp16
matmuls + softmax; output is fp16, DMAd per 512-col chunk on both rings.

Sharding: data-parallel over batch — core b handles batch element b.
"""

import numpy as np

B, C, N, H, W = 8, 128, 16, 56, 56
HW = H * W            # 3136
L = N * HW            # 50176
T = L // 128          # 392 chunks of 128 l-values
CW = 130              # chunk width in xt layout (128 cols + ones + pad)
NG = T // 4           # 98 fold-4 groups
# tile sizes balance per-DMA boundary overhead (~0.2us each on the single
# ring) against per-tile latency (a 48-chunk tile arrives in ~4.6us vs
# ~3.7us of PE work, so the in-order PE stream never starves into a HAM
# re-throttle); the last two tiles are tiny so the fold/xm tail is short
TILE_CHUNKS = [16, 16, 32, 48, 48, 48, 48, 48, 48, 24, 8, 8]
N_CORES = 8

# fp16 const-pack column layout
_WQ, _WK, _WV, _WO, _ID = 0, 128, 256, 384, 512
_BV, _BO, _BQ, _BK, _LBK = 640, 641, 642, 770, 898
_PACKW = 1026

_last_results = None  # BassKernelResults of the most recent run (for profiling)


def _ensure_axon_hooks_module():
    """bass_utils imports antenv.axon_hooks when BASS_TRACE is set; some
    images lack that module. Provide an inert registry so tracing degrades
    gracefully instead of raising."""
    import sys

    try:
        import antenv.axon_hooks  # noqa: F401
    except ImportError:
        import types

        try:
            import antenv
        except ImportError:
            return
        mod = types.ModuleType("antenv.axon_hooks")
        mod._hook = None
        mod.set_axon_ntff_profile_hook = lambda h: setattr(mod, "_hook", h)
        mod.get_axon_ntff_profile_hook = lambda: mod._hook
        sys.modules["antenv.axon_hooks"] = mod
        antenv.axon_hooks = mod


def _apply_env_patches():
    """Workarounds for this container's walrus build.

    1. Tile's end-of-kernel Drain aggregates every outstanding sem wait onto
       one CTRL instruction, but this walrus rejects >1 wait per instruction
       ("Too many sync wait commands"): re-emit surplus waits as single-wait
       nops (see _split_multi_waits, applied post-build).
    2. --enable-ldw-opt=true lets codegen skip redundant LDWEIGHTS reloads
       for consecutive matmuls sharing a stationary operand.
    """
    import concourse.mybir as mybir
    import concourse.bass_utils as bu
    from concourse.tile import TileContext
    from concourse.vector_clock import ScopedClock

    _ensure_axon_hooks_module()

    if not getattr(TileContext, "_drain_patch_applied", False):

        def _split_drain_and_barrier(self, tick_clock, wait_clock):
            # All end-of-kernel waits go on GpSimd — the engine that then
            # clears the semaphores — so the clear cannot pass an in-flight
            # producer. The two all-engine barriers are dropped: every
            # engine's stream simply ends, and the runtime's completion
            # signal requires all engines (including GpSimd) to halt.
            probe = self.nc.gpsimd.nop(nofuse=True)
            wait_clock.add_sem_waits(
                probe.ins, ScopedClock({None: tick_clock.global_clock})
            )
            si = probe.ins.sync_info
            waits = list(si.on_wait) if si is not None else []
            if len(waits) > 1:
                probe.ins.sync_info = mybir.SyncInfo(
                    on_wait=waits[:1], on_update=list(si.on_update)
                )
                for w in waits[1:]:
                    n = self.nc.gpsimd.nop(nofuse=True)
                    n.ins.sync_info = mybir.SyncInfo(on_wait=[w], on_update=[])
            assert self.sems is not None
            popped = self.nc._tile_sem_poison_stack.pop()
            assert popped is self._sem_poison
            self.nc.clear_and_free_semaphores(list(self.sems.allocated().values()))

        TileContext._drain_and_barrier = _split_drain_and_barrier
        TileContext._drain_patch_applied = True

    if not getattr(bu, "_ldw_opt_patch_applied", False):
        orig = bu.get_walrus_args

        def _walrus_args_ldw_opt(*a, **kw):
            return [
                arg.replace("--enable-ldw-opt=false", "--enable-ldw-opt=true")
                for arg in orig(*a, **kw)
            ]

        bu.get_walrus_args = _walrus_args_ldw_opt
        bu._ldw_opt_patch_applied = True


def _split_multi_waits(nc, max_waits=1):
    """Move surplus semaphore waits onto single-wait nops inserted just before
    the owning instruction on the same engine (the sequencer executes them in
    order, so the guarded instruction still issues only after all waits)."""
    import concourse.mybir as mybir

    k = 0
    for f in nc.m.functions:
        for b in f.blocks:
            il = list(b.instructions)
            new = []
            changed = False
            for inst in il:
                si = inst.sync_info
                waits = list(si.on_wait) if si is not None else []
                if len(waits) > max_waits:
                    changed = True
                    for w in waits[:-max_waits]:
                        nop = mybir.InstNoOp(name=f"Wsplit-{k}", ins=[], outs=[])
                        k += 1
                        nop.engine = inst.engine
                        nop.sync_info = mybir.SyncInfo(on_wait=[w], on_update=[])
                        new.append(nop)
                    inst.sync_info = mybir.SyncInfo(
                        on_wait=waits[-max_waits:], on_update=list(si.on_update)
                    )
                new.append(inst)
            if changed:
                b.instructions = new


def _hoist_first_dmas(nc, n=4):
    """Move the first wait-free DMA loads (first x tiles + the const packs)
    from the tile-context block into the entry block, ahead of the framework's
    engine-init barriers, so the HBM transfers overlap the ~7 us prologue."""
    for f in nc.m.functions:
        blocks = list(f.blocks)
        if len(blocks) < 2:
            continue
        entry, body = blocks[0], blocks[1]
        bil = list(body.instructions)
        dmas = []
        for i in bil:
            if i.opcode == "DMACopy":
                si = i.sync_info
                if si is None or not si.on_wait:
                    dmas.append(i)
                if len(dmas) >= n:
                    break
        if not dmas:
            continue
        picked = set(id(x) for x in dmas)
        body.instructions = [i for i in bil if id(i) not in picked]
        for k, i in enumerate(dmas):
            try:
                i.name = f"I-2-h{k}"
            except Exception:
                pass
        eil = list(entry.instructions)
        entry.instructions = eil[:1] + dmas + eil[1:]


def _window_pieces(w0):
    """Split the hw window [w0, w0+128) into pieces that neither wrap 3136 nor
    cross a 512-wide PSUM bank boundary. Returns (dst_hw, src_col, width)."""
    if w0 + 128 <= HW:
        segs = [(w0, 0, 128)]
    else:
        r = HW - w0
        segs = [(w0, 0, r), (0, r, 128 - r)]
    out = []
    for d, s, n in segs:
        while n > 0:
            m = min(n, 512 - (d % 512))
            out.append((d, s, m))
            d += m
            s += m
            n -= m
    return out


def _group_order():
    """Fold-4 group base chunks k in [0, NG), sorted so that low PSUM banks
    finish accumulating early (drains then overlap the input stream)."""
    keyed = []
    for k in range(NG):
        w0 = (128 * k) % HW
        minb = min(d // 512 for d, s, n in _window_pieces(w0))
        keyed.append((minb, w0, k))
    keyed.sort()
    return [k for _, _, k in keyed]


_GROUPS = _group_order()


def _build_nc():
    import concourse.bass as bass
    import concourse.mybir as mybir
    from concourse.tile import TileContext

    _apply_env_patches()

    f32, f16 = mybir.dt.float32, mybir.dt.float16
    nc = bass.Bass()

    xt = nc.dram_tensor("xt", [128, T * CW], f16, kind="ExternalInput")
    ic_d = nc.dram_tensor("ic16", [128, 128], f16, kind="ExternalInput")
    pk_d = nc.dram_tensor("pack", [128, _PACKW], f16, kind="ExternalInput")
    out_d = nc.dram_tensor("out", [128, HW], f16, kind="ExternalOutput")

    # per-bank write counts over the sorted fold groups; the last 4 groups
    # (the two 8-chunk tiles) skip DVE folding and write 4 raw-chunk
    # matmuls per window piece, so the tail never waits on fold adds
    writes_per_bank = [0] * 7
    for gi, k in enumerate(_GROUPS):
        mult = 4 if gi >= NG - 4 else 1
        for d, s, n in _window_pieces((128 * k) % HW):
            writes_per_bank[d // 512] += mult
    bank_width = [512] * 6 + [64]

    with TileContext(nc) as tc:
        with (
            tc.tile_pool(name="consts", bufs=1) as consts,
            tc.tile_pool(name="xtiles", bufs=1) as xtiles,
            tc.tile_pool(name="sbres", bufs=1) as sbres,
        ):
            # x tiles are all SBUF-resident; DMAs alternate between the two
            # HWDGE rings (sync / scalar) and never wait on compute.
            tiles = []
            t0s = []
            t0 = 0
            for j, ntc in enumerate(TILE_CHUNKS):
                tiles.append(
                    xtiles.tile([128, ntc * CW], f16, name=f"xt{j}", tag=f"xt{j}")
                )
                t0s.append(t0)
                t0 += ntc
            assert t0 == T

            # both HWDGE rings carry the x stream: each tile is DMAd as two
            # halves — sync (the earlier-starting ring) gets the first half,
            # scalar the second — so arrival order matches the in-order PE
            # consumption and the two rings sum to ~380 GB/s. The first two
            # tiles and the tiny last two ride whole on sync; consts warm
            # up scalar's ring.
            nc.sync.dma_start(out=tiles[0][:], in_=xt[:, 0 : TILE_CHUNKS[0] * CW])
            ic_sb = consts.tile([128, 128], f16)
            nc.scalar.dma_start(out=ic_sb[:], in_=ic_d[:])
            pk_sb = consts.tile([128, _PACKW], f16)
            nc.scalar.dma_start(out=pk_sb[:], in_=pk_d[:])
            nc.sync.dma_start(
                out=tiles[1][:],
                in_=xt[:, t0s[1] * CW : (t0s[1] + TILE_CHUNKS[1]) * CW],
            )
            for j in range(2, len(TILE_CHUNKS)):
                ntc = TILE_CHUNKS[j]
                base = t0s[j]
                if ntc <= 8:
                    nc.sync.dma_start(
                        out=tiles[j][:],
                        in_=xt[:, base * CW : (base + ntc) * CW],
                    )
                    continue
                h = (ntc * 15) // 32  # sync's share (balances const bytes)
                nc.sync.dma_start(
                    out=tiles[j][:, 0 : h * CW],
                    in_=xt[:, base * CW : (base + h) * CW],
                )
                nc.scalar.dma_start(
                    out=tiles[j][:, h * CW : ntc * CW],
                    in_=xt[:, (base + h) * CW : (base + ntc) * CW],
                )

            # preload the ACT Exp table while DMAs stream
            warm = sbres.tile([1, 1], f32)
            nc.vector.memset(warm[:], 0.0)
            nc.scalar.activation(
                out=warm[:], in_=warm[:],
                func=mybir.ActivationFunctionType.Exp, bias=0.0, scale=1.0,
            )
            # HAM warm-up: ~3.4us of dummy matmuls during the DMA prologue so
            # the PE clock is at 2.4 GHz before the first tile lands. Results
            # land in g_ps and are discarded by the real group's start=True.
            junk = sbres.tile([128, 128], f16, name="junk")
            nc.vector.memset(junk[:], 0.0)

            wqT_sb = pk_sb[:, _WQ : _WQ + 128]
            wkT_sb = pk_sb[:, _WK : _WK + 128]
            wv_sb = pk_sb[:, _WV : _WV + 128]
            woT_sb = pk_sb[:, _WO : _WO + 128]
            id_sb = pk_sb[:, _ID : _ID + 128]
            bv_sb = pk_sb[:, _BV : _BV + 1]
            bo_sb = pk_sb[:, _BO : _BO + 1]
            bq_sb = pk_sb[0:1, _BQ : _BQ + 128]
            bk_sb = pk_sb[0:1, _BK : _BK + 128]
            lbk_sb = pk_sb[0:1, _LBK : _LBK + 128]

            # ---- pass 1: [G|s] over all chunks; x_mean via 2-round DVE
            # fold of 4-chunk groups then an I/16 transpose-matmul per
            # folded chunk, one tile behind the G stream ----
            with (
                tc.tile_pool(name="folds", bufs=2) as folds,
                tc.tile_pool(name="folds2", bufs=3) as folds2,
                tc.tile_pool(name="ps1", bufs=1, space="PSUM") as ps1,
            ):
                # PSUM is bank-granular (8 x 512 fp32): [G|s] + 7 x_mean
                # banks fill all 8. The small tail tiles later reuse the
                # DRAINED xm banks as slices — safe because a start=True
                # clears has_written flags bank-wide, which only endangers
                # groups still accumulating in that bank, and each reused
                # bank hosts strictly sequential (drained-before-next-start)
                # groups.
                g_ps = ps1.tile([128, CW], f32)
                for _ in range(32):
                    nc.tensor.matmul(
                        g_ps[:, 0:128], lhsT=junk[:], rhs=junk[:],
                        start=True, stop=True, skip_group_check=True,
                    )
                xm_ps = [
                    ps1.tile([128, bank_width[k]], f32, name=f"xm{k}", tag=f"xm{k}")
                    for k in range(7)
                ]
                xm_sb = sbres.tile([128, HW], f16)
                gs_sb = sbres.tile([128, CW], f16)

                seen_per_bank = [0] * 7
                gpos = 0          # fold groups emitted (folded-chunk index)
                xm_done = 0       # folded chunks whose xm matmuls are emitted
                fold_bufs = {}    # tile j -> (r2 buffer, ngroups, gstart)

                ntiles = len(TILE_CHUNKS)

                def emit_xm(upto):
                    """Emit xm transpose matmuls for folded chunks [xm_done, upto);
                    drain a PSUM bank on its last write (ACT — keeps the DVE
                    FIFO clear for the fold adds and the tail drains). The
                    last 4 groups use 4 raw-chunk matmuls instead of a fold."""
                    nonlocal xm_done
                    for i in range(xm_done, upto):
                        if i >= NG - 4:
                            jt = ntiles - 2 if i < NG - 2 else ntiles - 1
                            loc = 4 * i - t0s[jt]
                            chs = [
                                tiles[jt][:, CW * (loc + m) : CW * (loc + m) + 128]
                                for m in range(4)
                            ]
                        else:
                            r2_sb, goff = None, None
                            # locate the tile buffer holding folded chunk i
                            for buf, ngr2, gst in fold_bufs.values():
                                if gst <= i < gst + ngr2:
                                    r2_sb, goff = buf, i - gst
                                    break
                            chs = [r2_sb[:, goff, 0:128]]
                        k = _GROUPS[i]
                        for ch in chs:
                            for d, s, n in _window_pieces((128 * k) % HW):
                                bk_i = d // 512
                                seen_per_bank[bk_i] += 1
                                nc.tensor.matmul(
                                    xm_ps[bk_i][:, d % 512 : d % 512 + n],
                                    lhsT=ch,
                                    rhs=ic_sb[:, s : s + n],
                                    start=(seen_per_bank[bk_i] == 1),
                                    stop=(
                                        seen_per_bank[bk_i]
                                        == writes_per_bank[bk_i]
                                    ),
                                )
                                if seen_per_bank[bk_i] == writes_per_bank[bk_i]:
                                    # bank complete: drain to SBUF now, off
                                    # the critical tail
                                    wdt = bank_width[bk_i]
                                    dst = xm_sb[:, 512 * bk_i : 512 * bk_i + wdt]
                                    nc.scalar.activation(
                                        out=dst, in_=xm_ps[bk_i][:, 0:wdt],
                                        func=mybir.ActivationFunctionType.Identity,
                                        bias=0.0, scale=1.0,
                                    )
                    xm_done = upto

                gend = []
                for j, ntc in enumerate(TILE_CHUNKS):
                    xt_sb = tiles[j]
                    # G | s accumulation over this tile's chunks
                    for i in range(ntc):
                        p = t0s[j] + i
                        nc.tensor.matmul(
                            g_ps[:],
                            lhsT=xt_sb[:, CW * i : CW * i + 128],
                            rhs=xt_sb[:, CW * i : CW * i + CW],
                            start=(p == 0),
                            stop=(p == T - 1),
                        )
                    ngr = ntc // 4
                    if j < ntiles - 2:
                        # fold rounds on DVE: 4 chunks -> 1
                        v4 = xt_sb[:].rearrange(
                            "q (g p two c) -> q g p two c", p=2, two=2, c=CW
                        )
                        r1_sb = folds.tile(
                            [128, ngr, 2, CW], f16, name=f"f1_{j}", tag="f1"
                        )
                        nc.vector.tensor_add(
                            out=r1_sb[:], in0=v4[:, :, :, 0, :],
                            in1=v4[:, :, :, 1, :],
                        )
                        r2_sb = folds2.tile(
                            [128, ngr, CW], f16, name=f"f2_{j}", tag="f2"
                        )
                        nc.vector.tensor_add(
                            out=r2_sb[:], in0=r1_sb[:, :, 0, :],
                            in1=r1_sb[:, :, 1, :],
                        )
                        fold_bufs[j] = (r2_sb, ngr, gpos)
                    gpos += ngr
                    gend.append(gpos)
                    # xm matmuls lag one tile so PE never waits on folds
                    if j > 0:
                        emit_xm(gend[j - 1])
                emit_xm(NG)

                # drain [G|s] (vector; PSUM -> SBUF fp16)
                nc.vector.tensor_copy(out=gs_sb[:], in_=g_ps[:])

                # ---- serial tail, small PSUM tiles allocated inside the
                # pass-1 pool (fits: 3266 + 769 <= 4096 fp32 columns) so the
                # first tail matmul does not wait on the ps1 pool barrier.
                # logits = Wq G Wk^T + (Wq s) bk^T + bq (Wk s)^T + L bq bk^T:
                # the G path (gw = G Wk^T) and the s path (rows = [s^T Wk^T |
                # s^T Wq^T]) run in parallel instead of chaining through V1.
                # tail tiles live in drained xm banks; per bank the groups
                # are strictly sequential: xm0 -> gw; xm1 -> lg then mt;
                # xm2 -> rows then cvec; xm3 -> u
                g_sb = gs_sb[:, 0:128]
                s_col = gs_sb[:, 128:129]
                lg_ps = xm_ps[1][:, 0:128]
                gw_ps = xm_ps[0][:, 0:128]
                u_ps = xm_ps[3][:, 0:128]
                mt_ps = xm_ps[1][:, 128:256]
                cv_ps = xm_ps[2][:, 256:257]

                nc.tensor.matmul(
                    lg_ps, lhsT=bq_sb, rhs=lbk_sb, start=True, stop=False,
                    skip_group_check=True,
                )
                # (Wk s)^T and (Wq s)^T as 1-partition rows (shared stationary)
                nc.tensor.matmul(
                    xm_ps[2][0:1, 0:128], lhsT=s_col, rhs=wkT_sb,
                    start=True, stop=True, skip_group_check=True,
                )
                nc.tensor.matmul(
                    xm_ps[2][0:1, 128:256], lhsT=s_col, rhs=wqT_sb,
                    start=True, stop=True, skip_group_check=True,
                )
                # gw = G Wk^T
                nc.tensor.matmul(
                    gw_ps, lhsT=g_sb, rhs=wkT_sb, start=True, stop=True,
                    skip_group_check=True,
                )
                rows_sb = sbres.tile([1, 256], f16)
                nc.vector.tensor_copy(out=rows_sb[:], in_=xm_ps[2][0:1, 0:256])
                kkrow_sb = rows_sb[:, 0:128]
                qsrow_sb = rows_sb[:, 128:256]
                gw_sb = sbres.tile([128, 128], f16)
                nc.vector.tensor_copy(out=gw_sb[:], in_=gw_ps)

                nc.tensor.matmul(
                    lg_ps, lhsT=bq_sb, rhs=kkrow_sb, start=False, stop=False,
                    skip_group_check=True,
                )
                nc.tensor.matmul(
                    lg_ps, lhsT=qsrow_sb, rhs=bk_sb, start=False, stop=False,
                    skip_group_check=True,
                )
                nc.tensor.matmul(
                    lg_ps, lhsT=wqT_sb, rhs=gw_sb[:], start=False, stop=True,
                    skip_group_check=True,
                )

                # softmax over the free axis (ACT only does the exp)
                negmax = sbres.tile([128, 1], f32)
                nc.vector.tensor_reduce(
                    out=negmax[:], in_=lg_ps, axis=mybir.AxisListType.X,
                    op=mybir.AluOpType.max, negate=True,
                )
                a_sb = sbres.tile([128, 128], f16)
                sumexp = sbres.tile([128, 1], f32)
                nc.scalar.activation(
                    out=a_sb[:], in_=lg_ps,
                    func=mybir.ActivationFunctionType.Exp,
                    bias=negmax[:], scale=1.0, accum_out=sumexp[:],
                )
                rec = sbres.tile([128, 1], f32)
                nc.vector.reciprocal(out=rec[:], in_=sumexp[:])
                nc.vector.tensor_scalar_mul(a_sb[:], a_sb[:], rec[:])

                # U = A^T Wo^T  [b, o]
                nc.tensor.matmul(
                    u_ps, lhsT=a_sb[:], rhs=woT_sb, start=True, stop=True,
                    skip_group_check=True,
                )
                u_sb = sbres.tile([128, 128], f16)
                nc.vector.tensor_copy(out=u_sb[:], in_=u_ps)

                # M^T = Wv^T A^T Wo^T ; P^T = M^T + I
                nc.tensor.matmul(
                    mt_ps, lhsT=wv_sb, rhs=u_sb[:], start=True, stop=True,
                    skip_group_check=True,
                )
                pt_sb = sbres.tile([128, 128], f16)
                nc.vector.tensor_add(out=pt_sb[:], in0=mt_ps, in1=id_sb)

                # cvec = U^T bv + bo  [o,1]
                nc.tensor.matmul(
                    cv_ps, lhsT=u_sb[:], rhs=bv_sb,
                    start=True, stop=True, skip_group_check=True,
                )
                cvec_sb = sbres.tile([128, 1], f32)
                nc.vector.scalar_tensor_tensor(
                    out=cvec_sb[:],
                    in0=cv_ps,
                    scalar=1.0,
                    in1=bo_sb,
                    op0=mybir.AluOpType.mult,
                    op1=mybir.AluOpType.add,
                )

            # ---- out = (M + I) x_mean + cvec (PSUM pool reuses the
            # drained pass-1 banks; its entry dep is satisfied by then) ----
            with tc.tile_pool(name="ps2", bufs=1, space="PSUM") as ps2:
                # 7 chunks of <=512 columns;
                # bias-adds alternate DVE/ACT; output DMAs are grouped into
                # four 2-chunk transfers (last one the tiny 64-col piece so
                # its HBM write receipt clears quickly), alternating rings
                out_sb = sbres.tile([128, HW], f16)
                oc_sizes = [512] * 6 + [64]
                dma_after = {1: (0, 1024), 3: (1024, 2048), 5: (2048, 3072),
                             6: (3072, HW)}
                off = 0
                for k, wdt in enumerate(oc_sizes):
                    oc_ps = ps2.tile(
                        [128, 512], f32, name=f"oc{k}", tag="oc", bufs=4
                    )
                    nc.tensor.matmul(
                        oc_ps[:, 0:wdt],
                        lhsT=pt_sb[:],
                        rhs=xm_sb[:, off : off + wdt],
                        start=True, stop=True,
                    )
                    ob = out_sb[:, off : off + wdt]
                    if k % 2 == 0:
                        nc.vector.tensor_scalar_add(
                            ob, oc_ps[:, 0:wdt], cvec_sb[:]
                        )
                    else:
                        nc.scalar.activation(
                            out=ob, in_=oc_ps[:, 0:wdt],
                            func=mybir.ActivationFunctionType.Identity,
                            bias=cvec_sb[:], scale=1.0,
                        )
                    off += wdt
                    if k in dma_after:
                        lo, hi = dma_after[k]
                        eng = nc.sync if k in (1, 5) else nc.scalar
                        eng.dma_start(
                            out=out_d[:, lo:hi], in_=out_sb[:, lo:hi]
                        )

    _split_multi_waits(nc)
    _hoist_first_dmas(nc)
    return nc


_cached_nc = None


def kernel(x, w_q, b_q, w_k, b_k, w_v, b_v, w_o, b_o):
    global _cached_nc, _last_results
    from concourse.bass_utils import run_bass_kernel_spmd

    if _cached_nc is None:
        _cached_nc = _build_nc()
    nc = _cached_nc

    x = np.asarray(x, np.float32)
    pack = np.zeros((128, _PACKW), np.float16)
    pack[:, _WQ : _WQ + 128] = np.asarray(w_q, np.float32).T.astype(np.float16)
    pack[:, _WK : _WK + 128] = np.asarray(w_k, np.float32).T.astype(np.float16)
    pack[:, _WV : _WV + 128] = np.asarray(w_v, np.float32).astype(np.float16)
    pack[:, _WO : _WO + 128] = np.asarray(w_o, np.float32).T.astype(np.float16)
    pack[:, _ID : _ID + 128] = np.eye(128, dtype=np.float16)
    pack[:, _BV] = np.asarray(b_v, np.float16)
    pack[:, _BO] = np.asarray(b_o, np.float16)
    pack[0, _BQ : _BQ + 128] = np.asarray(b_q, np.float16)
    pack[0, _BK : _BK + 128] = np.asarray(b_k, np.float16)
    pack[0, _LBK : _LBK + 128] = (float(L) * np.asarray(b_k, np.float64)).astype(
        np.float16
    )
    ic16 = np.ascontiguousarray((np.eye(128) / 16.0).astype(np.float16))

    # chunk order: fold groups of 4 (base k; members k, k+98, k+196, k+294
    # share an x_mean window), groups sorted so PSUM banks complete early
    order = np.empty(T, np.int64)
    for gi, k in enumerate(_GROUPS):
        for m in range(4):
            order[4 * gi + m] = k + NG * m
    in_maps = []
    for b in range(B):
        # xt[p, CW*t + c] = x[b, c, 128*t + p] for c < 128; ones at c == 128
        xb = x[b].reshape(C, T, 128)
        xt_b = np.zeros((128, T, CW), np.float16)
        xt_b[:, :, :128] = xb.transpose(2, 1, 0)[:, order, :].astype(np.float16)
        xt_b[:, :, 128] = np.float16(1.0)
        in_maps.append(
            {"xt": xt_b.reshape(128, T * CW), "ic16": ic16, "pack": pack}
        )

    res = run_bass_kernel_spmd(nc, in_maps, list(range(N_CORES)))
    _last_results = res

    out = np.empty((B, C, H, W), np.float32)
    for b in range(B):
        out[b] = res.results[b]["out"].astype(np.float32).reshape(C, H, W)
    return out


# revision 32
# speedup vs baseline: 1.0531x; 1.0531x over previous
"""Channel self-attention module (CSMA) on 8 Trainium2 NeuronCores.

Math: with x [B,C,N,H,W], C==HID==OUT==128, L=N*H*W, the module is
    q = Wq x + bq ; k = Wk x + bk ; v = Wv x + bv          (per-batch [C,L])
    A = softmax(q k^T)                                     ([C,C], rows)
    out = Wo (A v) + bo + x ; result = mean_N(out)         ([C,H*W])

Everything except the softmax is linear in x, so per batch only two small
sufficient statistics of x are needed:
    G = x x^T  [C,C]   and   s = x 1_L  [C]
    logits = Wq G Wk^T + (Wq s) bk^T + bq (Wk s)^T + L bq bk^T
    A = softmax(logits)
    result = (Wo A Wv + I) x_mean + (Wo A bv + bo)
where x_mean = mean over N of x (shape [C, H*W]).

The input stream (13 MB/core fp16) saturates the per-core HBM share
(~358 GB/s) and is the wall-clock floor, so the schedule keeps all other
engines hidden underneath it: x is pre-transposed on the host to l-major
fp16 chunks [128l, 130] (128 cols + ones + pad) and DMAd into SBUF fully
resident (no buffer recycling -> DMA never stalls on compute). Per tile,
PE accumulates [G|s]; DVE folds groups of 4 chunks that share an x_mean
window (k, k+98, k+196, k+294 -- 98*128 = 4*3136) in two add rounds; PE
transposes each folded chunk into the x_mean PSUM window via an I/16
matmul (one tile behind the G stream). The host orders fold groups by
PSUM bank so banks complete early and drain mid-stream (ACT/DVE
alternating). Pass 2 is a short serial tail of [128,128]-scale fp16
matmuls + softmax; output is fp16, DMAd per 512-col chunk on both rings.

Sharding: data-parallel over batch — core b handles batch element b.
"""

import numpy as np

B, C, N, H, W = 8, 128, 16, 56, 56
HW = H * W            # 3136
L = N * HW            # 50176
T = L // 128          # 392 chunks of 128 l-values
CW = 130              # chunk width in xt layout (128 cols + ones + pad)
NG = T // 4           # 98 fold-4 groups
# tile sizes balance per-DMA boundary overhead (~0.2us each on the single
# ring) against per-tile latency (a 48-chunk tile arrives in ~4.6us vs
# ~3.7us of PE work, so the in-order PE stream never starves into a HAM
# re-throttle); the last two tiles are tiny so the fold/xm tail is short
TILE_CHUNKS = [16, 16, 32, 48, 48, 48, 48, 48, 48, 24, 8, 8]
N_CORES = 8

# fp16 const-pack column layout
_WQ, _WK, _WV, _WO, _ID = 0, 128, 256, 384, 512
_BV, _BO, _BQ, _BK, _LBK = 640, 641, 642, 770, 898
_PACKW = 1026

_last_results = None  # BassKernelResults of the most recent run (for profiling)


def _ensure_axon_hooks_module():
    """bass_utils imports antenv.axon_hooks when BASS_TRACE is set; some
    images lack that module. Provide an inert registry so tracing degrades
    gracefully instead of raising."""
    import sys

    try:
        import antenv.axon_hooks  # noqa: F401
    except ImportError:
        import types

        try:
            import antenv
        except ImportError:
            return
        mod = types.ModuleType("antenv.axon_hooks")
        mod._hook = None
        mod.set_axon_ntff_profile_hook = lambda h: setattr(mod, "_hook", h)
        mod.get_axon_ntff_profile_hook = lambda: mod._hook
        sys.modules["antenv.axon_hooks"] = mod
        antenv.axon_hooks = mod


def _apply_env_patches():
    """Workarounds for this container's walrus build.

    1. Tile's end-of-kernel Drain aggregates every outstanding sem wait onto
       one CTRL instruction, but this walrus rejects >1 wait per instruction
       ("Too many sync wait commands"): re-emit surplus waits as single-wait
       nops (see _split_multi_waits, applied post-build).
    2. --enable-ldw-opt=true lets codegen skip redundant LDWEIGHTS reloads
       for consecutive matmuls sharing a stationary operand.
    """
    import concourse.mybir as mybir
    import concourse.bass_utils as bu
    from concourse.tile import TileContext
    from concourse.vector_clock import ScopedClock

    _ensure_axon_hooks_module()

    if not getattr(TileContext, "_drain_patch_applied", False):

        def _split_drain_and_barrier(self, tick_clock, wait_clock):
            # All end-of-kernel waits go on GpSimd — the engine that then
            # clears the semaphores — so the clear cannot pass an in-flight
            # producer. The two all-engine barriers are dropped: every
            # engine's stream simply ends, and the runtime's completion
            # signal requires all engines (including GpSimd) to halt.
            probe = self.nc.gpsimd.nop(nofuse=True)
            wait_clock.add_sem_waits(
                probe.ins, ScopedClock({None: tick_clock.global_clock})
            )
            si = probe.ins.sync_info
            waits = list(si.on_wait) if si is not None else []
            if len(waits) > 1:
                probe.ins.sync_info = mybir.SyncInfo(
                    on_wait=waits[:1], on_update=list(si.on_update)
                )
                for w in waits[1:]:
                    n = self.nc.gpsimd.nop(nofuse=True)
                    n.ins.sync_info = mybir.SyncInfo(on_wait=[w], on_update=[])
            assert self.sems is not None
            popped = self.nc._tile_sem_poison_stack.pop()
            assert popped is self._sem_poison
            self.nc.clear_and_free_semaphores(list(self.sems.allocated().values()))

        TileContext._drain_and_barrier = _split_drain_and_barrier
        TileContext._drain_patch_applied = True

    if not getattr(bu, "_ldw_opt_patch_applied", False):
        orig = bu.get_walrus_args

        def _walrus_args_ldw_opt(*a, **kw):
            return [
                arg.replace("--enable-ldw-opt=false", "--enable-ldw-opt=true")
                for arg in orig(*a, **kw)
            ]

        bu.get_walrus_args = _walrus_args_ldw_opt
        bu._ldw_opt_patch_applied = True


def _split_multi_waits(nc, max_waits=1):
    """Move surplus semaphore waits onto single-wait nops inserted just before
    the owning instruction on the same engine (the sequencer executes them in
    order, so the guarded instruction still issues only after all waits)."""
    import concourse.mybir as mybir

    k = 0
    for f in nc.m.functions:
        for b in f.blocks:
            il = list(b.instructions)
            new = []
            changed = False
            for inst in il:
                si = inst.sync_info
                waits = list(si.on_wait) if si is not None else []
                if len(waits) > max_waits:
                    changed = True
                    for w in waits[:-max_waits]:
                        nop = mybir.InstNoOp(name=f"Wsplit-{k}", ins=[], outs=[])
                        k += 1
                        nop.engine = inst.engine
                        nop.sync_info = mybir.SyncInfo(on_wait=[w], on_update=[])
                        new.append(nop)
                    inst.sync_info = mybir.SyncInfo(
                        on_wait=waits[-max_waits:], on_update=list(si.on_update)
                    )
                new.append(inst)
            if changed:
                b.instructions = new


def _hoist_first_dmas(nc, n=4):
    """Move the first wait-free DMA loads (first x tiles + the const packs)
    from the tile-context block into the entry block, ahead of the framework's
    engine-init barriers, so the HBM transfers overlap the ~7 us prologue."""
    for f in nc.m.functions:
        blocks = list(f.blocks)
        if len(blocks) < 2:
            continue
        entry, body = blocks[0], blocks[1]
        bil = list(body.instructions)
        dmas = []
        for i in bil:
            if i.opcode == "DMACopy":
                si = i.sync_info
                if si is None or not si.on_wait:
                    dmas.append(i)
                if len(dmas) >= n:
                    break
        if not dmas:
            continue
        picked = set(id(x) for x in dmas)
        body.instructions = [i for i in bil if id(i) not in picked]
        for k, i in enumerate(dmas):
            try:
                i.name = f"I-2-h{k}"
            except Exception:
                pass
        eil = list(entry.instructions)
        entry.instructions = eil[:1] + dmas + eil[1:]


def _window_pieces(w0):
    """Split the hw window [w0, w0+128) into pieces that neither wrap 3136 nor
    cross a 512-wide PSUM bank boundary. Returns (dst_hw, src_col, width)."""
    if w0 + 128 <= HW:
        segs = [(w0, 0, 128)]
    else:
        r = HW - w0
        segs = [(w0, 0, r), (0, r, 128 - r)]
    out = []
    for d, s, n in segs:
        while n > 0:
            m = min(n, 512 - (d % 512))
            out.append((d, s, m))
            d += m
            s += m
            n -= m
    return out


def _group_order():
    """Fold-4 group base chunks k in [0, NG), sorted so that low PSUM banks
    finish accumulating early (drains then overlap the input stream)."""
    keyed = []
    for k in range(NG):
        w0 = (128 * k) % HW
        minb = min(d // 512 for d, s, n in _window_pieces(w0))
        keyed.append((minb, w0, k))
    keyed.sort()
    return [k for _, _, k in keyed]


_GROUPS = _group_order()


def _build_nc():
    import concourse.bass as bass
    import concourse.mybir as mybir
    from concourse.tile import TileContext

    _apply_env_patches()

    f32, f16 = mybir.dt.float32, mybir.dt.float16
    nc = bass.Bass()

    xt = nc.dram_tensor("xt", [128, T * CW], f16, kind="ExternalInput")
    ic_d = nc.dram_tensor("ic16", [128, 128], f16, kind="ExternalInput")
    pk_d = nc.dram_tensor("pack", [128, _PACKW], f16, kind="ExternalInput")
    out_d = nc.dram_tensor("out", [128, HW], f16, kind="ExternalOutput")

    # per-bank write counts over the sorted fold groups; the last 4 groups
    # (the two 8-chunk tiles) skip DVE folding and write 4 raw-chunk
    # matmuls per window piece, so the tail never waits on fold adds
    writes_per_bank = [0] * 7
    for gi, k in enumerate(_GROUPS):
        mult = 4 if gi >= NG - 4 else 1
        for d, s, n in _window_pieces((128 * k) % HW):
            writes_per_bank[d // 512] += mult
    bank_width = [512] * 6 + [64]

    with TileContext(nc) as tc:
        with (
            tc.tile_pool(name="consts", bufs=1) as consts,
            tc.tile_pool(name="xtiles", bufs=1) as xtiles,
            tc.tile_pool(name="sbres", bufs=1) as sbres,
        ):
            # x tiles are all SBUF-resident; DMAs alternate between the two
            # HWDGE rings (sync / scalar) and never wait on compute.
            tiles = []
            t0s = []
            t0 = 0
            for j, ntc in enumerate(TILE_CHUNKS):
                tiles.append(
                    xtiles.tile([128, ntc * CW], f16, name=f"xt{j}", tag=f"xt{j}")
                )
                t0s.append(t0)
                t0 += ntc
            assert t0 == T

            # one ring (sync) carries the whole x stream in arrival order ==
            # consumption order (a single queue drives all 16 SDMA engines;
            # two uncoordinated rings deliver out of order and starve the
            # in-order PE stream); scalar's ring carries consts and later
            # the output, keeping ACT's istream clear of DMA waits
            nc.sync.dma_start(out=tiles[0][:], in_=xt[:, 0 : TILE_CHUNKS[0] * CW])
            ic_sb = consts.tile([128, 128], f16)
            nc.scalar.dma_start(out=ic_sb[:], in_=ic_d[:])
            pk_sb = consts.tile([128, _PACKW], f16)
            nc.scalar.dma_start(out=pk_sb[:], in_=pk_d[:])
            nc.sync.dma_start(
                out=tiles[1][:],
                in_=xt[:, t0s[1] * CW : (t0s[1] + TILE_CHUNKS[1]) * CW],
            )
            for j in range(2, len(TILE_CHUNKS)):
                nc.sync.dma_start(
                    out=tiles[j][:],
                    in_=xt[:, t0s[j] * CW : (t0s[j] + TILE_CHUNKS[j]) * CW],
                )

            # preload the ACT Exp table while DMAs stream
            warm = sbres.tile([1, 1], f32)
            nc.vector.memset(warm[:], 0.0)
            nc.scalar.activation(
                out=warm[:], in_=warm[:],
                func=mybir.ActivationFunctionType.Exp, bias=0.0, scale=1.0,
            )
            # HAM warm-up: ~3.4us of dummy matmuls during the DMA prologue so
            # the PE clock is at 2.4 GHz before the first tile lands. Results
            # land in g_ps and are discarded by the real group's start=True.
            junk = sbres.tile([128, 128], f16, name="junk")
            nc.vector.memset(junk[:], 0.0)

            wqT_sb = pk_sb[:, _WQ : _WQ + 128]
            wkT_sb = pk_sb[:, _WK : _WK + 128]
            wv_sb = pk_sb[:, _WV : _WV + 128]
            woT_sb = pk_sb[:, _WO : _WO + 128]
            id_sb = pk_sb[:, _ID : _ID + 128]
            bv_sb = pk_sb[:, _BV : _BV + 1]
            bo_sb = pk_sb[:, _BO : _BO + 1]
            bq_sb = pk_sb[0:1, _BQ : _BQ + 128]
            bk_sb = pk_sb[0:1, _BK : _BK + 128]
            lbk_sb = pk_sb[0:1, _LBK : _LBK + 128]

            # ---- pass 1: [G|s] over all chunks; x_mean via 2-round DVE
            # fold of 4-chunk groups then an I/16 transpose-matmul per
            # folded chunk, one tile behind the G stream ----
            with (
                tc.tile_pool(name="folds", bufs=2) as folds,
                tc.tile_pool(name="folds2", bufs=3) as folds2,
                tc.tile_pool(name="ps1", bufs=1, space="PSUM") as ps1,
            ):
                # PSUM is bank-granular (8 x 512 fp32): [G|s] + 7 x_mean
                # banks fill all 8. The small tail tiles later reuse the
                # DRAINED xm banks as slices — safe because a start=True
                # clears has_written flags bank-wide, which only endangers
                # groups still accumulating in that bank, and each reused
                # bank hosts strictly sequential (drained-before-next-start)
                # groups.
                g_ps = ps1.tile([128, CW], f32)
                for _ in range(32):
                    nc.tensor.matmul(
                        g_ps[:, 0:128], lhsT=junk[:], rhs=junk[:],
                        start=True, stop=True, skip_group_check=True,
                    )
                xm_ps = [
                    ps1.tile([128, bank_width[k]], f32, name=f"xm{k}", tag=f"xm{k}")
                    for k in range(7)
                ]
                xm_sb = sbres.tile([128, HW], f16)
                gs_sb = sbres.tile([128, CW], f16)

                seen_per_bank = [0] * 7
                gpos = 0          # fold groups emitted (folded-chunk index)
                xm_done = 0       # folded chunks whose xm matmuls are emitted
                fold_bufs = {}    # tile j -> (r2 buffer, ngroups, gstart)

                ntiles = len(TILE_CHUNKS)

                def emit_xm(upto):
                    """Emit xm transpose matmuls for folded chunks [xm_done, upto);
                    drain a PSUM bank on its last write (ACT — keeps the DVE
                    FIFO clear for the fold adds and the tail drains). The
                    last 4 groups use 4 raw-chunk matmuls instead of a fold."""
                    nonlocal xm_done
                    for i in range(xm_done, upto):
                        if i >= NG - 4:
                            jt = ntiles - 2 if i < NG - 2 else ntiles - 1
                            loc = 4 * i - t0s[jt]
                            chs = [
                                tiles[jt][:, CW * (loc + m) : CW * (loc + m) + 128]
                                for m in range(4)
                            ]
                        else:
                            r2_sb, goff = None, None
                            # locate the tile buffer holding folded chunk i
                            for buf, ngr2, gst in fold_bufs.values():
                                if gst <= i < gst + ngr2:
                                    r2_sb, goff = buf, i - gst
                                    break
                            chs = [r2_sb[:, goff, 0:128]]
                        k = _GROUPS[i]
                        for ch in chs:
                            for d, s, n in _window_pieces((128 * k) % HW):
                                bk_i = d // 512
                                seen_per_bank[bk_i] += 1
                                nc.tensor.matmul(
                                    xm_ps[bk_i][:, d % 512 : d % 512 + n],
                                    lhsT=ch,
                                    rhs=ic_sb[:, s : s + n],
                                    start=(seen_per_bank[bk_i] == 1),
                                    stop=(
                                        seen_per_bank[bk_i]
                                        == writes_per_bank[bk_i]
                                    ),
                                )
                                if seen_per_bank[bk_i] == writes_per_bank[bk_i]:
                                    # bank complete: drain to SBUF now, off
                                    # the critical tail
                                    wdt = bank_width[bk_i]
                                    dst = xm_sb[:, 512 * bk_i : 512 * bk_i + wdt]
                                    nc.scalar.activation(
                                        out=dst, in_=xm_ps[bk_i][:, 0:wdt],
                                        func=mybir.ActivationFunctionType.Identity,
                                        bias=0.0, scale=1.0,
                                    )
                    xm_done = upto

                gend = []
                for j, ntc in enumerate(TILE_CHUNKS):
                    xt_sb = tiles[j]
                    # G | s accumulation over this tile's chunks
                    for i in range(ntc):
                        p = t0s[j] + i
                        nc.tensor.matmul(
                            g_ps[:],
                            lhsT=xt_sb[:, CW * i : CW * i + 128],
                            rhs=xt_sb[:, CW * i : CW * i + CW],
                            start=(p == 0),
                            stop=(p == T - 1),
                        )
                    ngr = ntc // 4
                    if j < ntiles - 2:
                        # fold rounds on DVE: 4 chunks -> 1
                        v4 = xt_sb[:].rearrange(
                            "q (g p two c) -> q g p two c", p=2, two=2, c=CW
                        )
                        r1_sb = folds.tile(
                            [128, ngr, 2, CW], f16, name=f"f1_{j}", tag="f1"
                        )
                        nc.vector.tensor_add(
                            out=r1_sb[:], in0=v4[:, :, :, 0, :],
                            in1=v4[:, :, :, 1, :],
                        )
                        r2_sb = folds2.tile(
                            [128, ngr, CW], f16, name=f"f2_{j}", tag="f2"
                        )
                        nc.vector.tensor_add(
                            out=r2_sb[:], in0=r1_sb[:, :, 0, :],
                            in1=r1_sb[:, :, 1, :],
                        )
                        fold_bufs[j] = (r2_sb, ngr, gpos)
                    gpos += ngr
                    gend.append(gpos)
                    # xm matmuls lag one tile so PE never waits on folds
                    if j > 0:
                        emit_xm(gend[j - 1])
                emit_xm(NG)

                # drain [G|s] (vector; PSUM -> SBUF fp16)
                nc.vector.tensor_copy(out=gs_sb[:], in_=g_ps[:])

                # ---- serial tail, small PSUM tiles allocated inside the
                # pass-1 pool (fits: 3266 + 769 <= 4096 fp32 columns) so the
                # first tail matmul does not wait on the ps1 pool barrier.
                # logits = Wq G Wk^T + (Wq s) bk^T + bq (Wk s)^T + L bq bk^T:
                # the G path (gw = G Wk^T) and the s path (rows = [s^T Wk^T |
                # s^T Wq^T]) run in parallel instead of chaining through V1.
                # tail tiles live in drained xm banks; per bank the groups
                # are strictly sequential: xm0 -> gw; xm1 -> lg then mt;
                # xm2 -> rows then cvec; xm3 -> u
                g_sb = gs_sb[:, 0:128]
                s_col = gs_sb[:, 128:129]
                lg_ps = xm_ps[1][:, 0:128]
                gw_ps = xm_ps[0][:, 0:128]
                u_ps = xm_ps[3][:, 0:128]
                mt_ps = xm_ps[1][:, 128:256]
                cv_ps = xm_ps[2][:, 256:257]

                nc.tensor.matmul(
                    lg_ps, lhsT=bq_sb, rhs=lbk_sb, start=True, stop=False,
                    skip_group_check=True,
                )
                # (Wk s)^T and (Wq s)^T as 1-partition rows (shared stationary)
                nc.tensor.matmul(
                    xm_ps[2][0:1, 0:128], lhsT=s_col, rhs=wkT_sb,
                    start=True, stop=True, skip_group_check=True,
                )
                nc.tensor.matmul(
                    xm_ps[2][0:1, 128:256], lhsT=s_col, rhs=wqT_sb,
                    start=True, stop=True, skip_group_check=True,
                )
                # gw = G Wk^T
                nc.tensor.matmul(
                    gw_ps, lhsT=g_sb, rhs=wkT_sb, start=True, stop=True,
                    skip_group_check=True,
                )
                rows_sb = sbres.tile([1, 256], f16)
                nc.vector.tensor_copy(out=rows_sb[:], in_=xm_ps[2][0:1, 0:256])
                kkrow_sb = rows_sb[:, 0:128]
                qsrow_sb = rows_sb[:, 128:256]
                gw_sb = sbres.tile([128, 128], f16)
                nc.vector.tensor_copy(out=gw_sb[:], in_=gw_ps)

                nc.tensor.matmul(
                    lg_ps, lhsT=bq_sb, rhs=kkrow_sb, start=False, stop=False,
                    skip_group_check=True,
                )
                nc.tensor.matmul(
                    lg_ps, lhsT=qsrow_sb, rhs=bk_sb, start=False, stop=False,
                    skip_group_check=True,
                )
                nc.tensor.matmul(
                    lg_ps, lhsT=wqT_sb, rhs=gw_sb[:], start=False, stop=True,
                    skip_group_check=True,
                )

                # softmax over the free axis (ACT only does the exp)
                negmax = sbres.tile([128, 1], f32)
                nc.vector.tensor_reduce(
                    out=negmax[:], in_=lg_ps, axis=mybir.AxisListType.X,
                    op=mybir.AluOpType.max, negate=True,
                )
                a_sb = sbres.tile([128, 128], f16)
                sumexp = sbres.tile([128, 1], f32)
                nc.scalar.activation(
                    out=a_sb[:], in_=lg_ps,
                    func=mybir.ActivationFunctionType.Exp,
                    bias=negmax[:], scale=1.0, accum_out=sumexp[:],
                )
                rec = sbres.tile([128, 1], f32)
                nc.vector.reciprocal(out=rec[:], in_=sumexp[:])
                nc.vector.tensor_scalar_mul(a_sb[:], a_sb[:], rec[:])

                # U = A^T Wo^T  [b, o]
                nc.tensor.matmul(
                    u_ps, lhsT=a_sb[:], rhs=woT_sb, start=True, stop=True,
                    skip_group_check=True,
                )
                u_sb = sbres.tile([128, 128], f16)
                nc.vector.tensor_copy(out=u_sb[:], in_=u_ps)

                # M^T = Wv^T A^T Wo^T ; P^T = M^T + I
                nc.tensor.matmul(
                    mt_ps, lhsT=wv_sb, rhs=u_sb[:], start=True, stop=True,
                    skip_group_check=True,
                )
                pt_sb = sbres.tile([128, 128], f16)
                nc.vector.tensor_add(out=pt_sb[:], in0=mt_ps, in1=id_sb)

                # cvec = U^T bv + bo  [o,1]
                nc.tensor.matmul(
                    cv_ps, lhsT=u_sb[:], rhs=bv_sb,
                    start=True, stop=True, skip_group_check=True,
                )
                cvec_sb = sbres.tile([128, 1], f32)
                nc.vector.scalar_tensor_tensor(
                    out=cvec_sb[:],
                    in0=cv_ps,
                    scalar=1.0,
                    in1=bo_sb,
                    op0=mybir.AluOpType.mult,
                    op1=mybir.AluOpType.add,
                )

            # ---- out = (M + I) x_mean + cvec (PSUM pool reuses the
            # drained pass-1 banks; its entry dep is satisfied by then) ----
            with tc.tile_pool(name="ps2", bufs=1, space="PSUM") as ps2:
                # 7 chunks of <=512 columns;
                # bias-adds alternate DVE/ACT; output DMAs are grouped into
                # four 2-chunk transfers (last one the tiny 64-col piece so
                # its HBM write receipt clears quickly), alternating rings
                out_sb = sbres.tile([128, HW], f16)
                oc_sizes = [512] * 6 + [64]
                dma_after = {1: (0, 1024), 3: (1024, 2048), 5: (2048, 3072),
                             6: (3072, HW)}
                off = 0
                for k, wdt in enumerate(oc_sizes):
                    oc_ps = ps2.tile(
                        [128, 512], f32, name=f"oc{k}", tag="oc", bufs=4
                    )
                    nc.tensor.matmul(
                        oc_ps[:, 0:wdt],
                        lhsT=pt_sb[:],
                        rhs=xm_sb[:, off : off + wdt],
                        start=True, stop=True,
                    )
                    ob = out_sb[:, off : off + wdt]
                    if k % 2 == 0:
                        nc.vector.tensor_scalar_add(
                            ob, oc_ps[:, 0:wdt], cvec_sb[:]
                        )
                    else:
                        nc.scalar.activation(
                            out=ob, in_=oc_ps[:, 0:wdt],
                            func=mybir.ActivationFunctionType.Identity,
                            bias=cvec_sb[:], scale=1.0,
                        )
                    off += wdt
                    if k in dma_after:
                        lo, hi = dma_after[k]
                        eng = nc.sync if k in (1, 5) else nc.scalar
                        eng.dma_start(
                            out=out_d[:, lo:hi], in_=out_sb[:, lo:hi]
                        )

    _split_multi_waits(nc)
    _hoist_first_dmas(nc)
    return nc


_cached_nc = None


def kernel(x, w_q, b_q, w_k, b_k, w_v, b_v, w_o, b_o):
    global _cached_nc, _last_results
    from concourse.bass_utils import run_bass_kernel_spmd

    if _cached_nc is None:
        _cached_nc = _build_nc()
    nc = _cached_nc

    x = np.asarray(x, np.float32)
    pack = np.zeros((128, _PACKW), np.float16)
    pack[:, _WQ : _WQ + 128] = np.asarray(w_q, np.float32).T.astype(np.float16)
    pack[:, _WK : _WK + 128] = np.asarray(w_k, np.float32).T.astype(np.float16)
    pack[:, _WV : _WV + 128] = np.asarray(w_v, np.float32).astype(np.float16)
    pack[:, _WO : _WO + 128] = np.asarray(w_o, np.float32).T.astype(np.float16)
    pack[:, _ID : _ID + 128] = np.eye(128, dtype=np.float16)
    pack[:, _BV] = np.asarray(b_v, np.float16)
    pack[:, _BO] = np.asarray(b_o, np.float16)
    pack[0, _BQ : _BQ + 128] = np.asarray(b_q, np.float16)
    pack[0, _BK : _BK + 128] = np.asarray(b_k, np.float16)
    pack[0, _LBK : _LBK + 128] = (float(L) * np.asarray(b_k, np.float64)).astype(
        np.float16
    )
    ic16 = np.ascontiguousarray((np.eye(128) / 16.0).astype(np.float16))

    # chunk order: fold groups of 4 (base k; members k, k+98, k+196, k+294
    # share an x_mean window), groups sorted so PSUM banks complete early
    order = np.empty(T, np.int64)
    for gi, k in enumerate(_GROUPS):
        for m in range(4):
            order[4 * gi + m] = k + NG * m
    in_maps = []
    for b in range(B):
        # xt[p, CW*t + c] = x[b, c, 128*t + p] for c < 128; ones at c == 128
        xb = x[b].reshape(C, T, 128)
        xt_b = np.zeros((128, T, CW), np.float16)
        xt_b[:, :, :128] = xb.transpose(2, 1, 0)[:, order, :].astype(np.float16)
        xt_b[:, :, 128] = np.float16(1.0)
        in_maps.append(
            {"xt": xt_b.reshape(128, T * CW), "ic16": ic16, "pack": pack}
        )

    res = run_bass_kernel_spmd(nc, in_maps, list(range(N_CORES)))
    _last_results = res

    out = np.empty((B, C, H, W), np.float32)
    for b in range(B):
        out[b] = res.results[b]["out"].astype(np.float32).reshape(C, H, W)
    return out
